# revision 37
# baseline (speedup 1.0000x reference)
"""Trainium2 Bass kernel for nn_DynamicConv (dense_cnn).

out[i, j, co, h, w] = sum_k (conv_k(x_i)[co, h, w] + b_k[co]) * attn[j, k]
attn = softmax(softmax(MLP(meanpool(x)), k) / TAU, k)

Sharding: data-parallel over batch i across 8 cores.  Each core convolves its
own sample (9 shifted bf16 matmuls over a zero-padded image, contraction =
CIN=128) and computes the full [B, K] attention matrix locally: every core
loads all 8 x-slices (bf16, cheap on the 16-engine striped DMA ring), mean-
pools them on DVE, and runs the tiny MLP + double softmax itself — no
collective at all.  The cross-batch blend is a block-diagonal bf16 matmul per
16-channel group (contraction 64 = k4 x co16 in the partition-sliced rhs
half cs[64u:64u+64], M = 128 = j8 x co16), with blends interleaved between
conv tiles so PSUM evictions and output DMA spread across the whole run and
only blend7 trails the last conv.

All matmul operands are bf16 (PE full rate, 1 cyc/row; fp32-HIGH measured
~2x slower); PSUM accumulates fp32; the output slab is stored bf16 and
widened to fp32 on the host (rel-err budget ~3e-3 total).
"""

import sys

import numpy as np

if "/opt/trn_rl_repo" not in sys.path:
    sys.path.insert(0, "/opt/trn_rl_repo")

import ml_dtypes

import concourse.bacc as bacc
import concourse.bass as bass
import concourse.mybir as mybir
import concourse.tile as tile

F32 = mybir.dt.float32
BF16 = mybir.dt.bfloat16
AF = mybir.ActivationFunctionType
AX = mybir.AxisListType
ALU = mybir.AluOpType

B = 8
CIN = 128
COUT = 256
K = 4
KS = 3
HW = 48
HW2 = HW * HW          # 2304
WP = HW + 2            # 50 (padded)
HID = 256
TAU = 30.0
NCORES = 8

ROW_GROUPS = [(0, 10), (10, 10), (20, 10), (30, 10), (40, 8)]
CHUNKS = [(0, 512), (512, 512), (1024, 512), (1536, 512), (2048, 256)]


def build_nc():
    nc = bacc.Bacc("TRN2", debug=False, num_devices=NCORES)

    xi = nc.dram_tensor("xi", [CIN, HW2], BF16, kind="ExternalInput").ap()
    xall = nc.dram_tensor("xall", [CIN, B * HW2], BF16, kind="ExternalInput").ap()
    # [ci, t, tap, p] flattened; p = c*4 + k encodes (co = 32 t + c, k)
    wconv = nc.dram_tensor(
        "wconv", [CIN, 8 * 9 * 128], BF16, kind="ExternalInput"
    ).ap()
    bconv = nc.dram_tensor("bconv", [128, 8], F32, kind="ExternalInput").ap()
    w1t = nc.dram_tensor("w1t", [CIN, HID], BF16, kind="ExternalInput").ap()
    b1c = nc.dram_tensor("b1c", [128, 2], F32, kind="ExternalInput").ap()
    w2t = nc.dram_tensor("w2t", [128, 2 * K], BF16, kind="ExternalInput").ap()
    b2r = nc.dram_tensor("b2r", [1, K], BF16, kind="ExternalInput").ap()
    ident8 = nc.dram_tensor("ident8", [B, B], BF16, kind="ExternalInput").ap()
    one18 = nc.dram_tensor("one18", [1, B], BF16, kind="ExternalInput").ap()
    out = nc.dram_tensor("out", [B, COUT, HW2], BF16, kind="ExternalOutput").ap()

    with tile.TileContext(nc, num_cores=NCORES) as tc:
        with (
            tc.tile_pool(name="const", bufs=1) as const,
            tc.tile_pool(name="csb", bufs=8) as csb_pool,
            tc.tile_pool(name="osb", bufs=8) as osb_pool,
            tc.tile_pool(name="psA", bufs=3, space="PSUM") as psA,
            tc.tile_pool(name="psB", bufs=4, space="PSUM") as psB,
            tc.tile_pool(name="psM", bufs=1, space="PSUM") as psM,
        ):
            # pre-warm the ACT function tables (1.3us each if loaded lazily
            # inside the latency-critical chains)
            zc = const.tile([128, 1], F32)
            nc.gpsimd.memset(zc[:], 0.0)
            actw = const.tile([128, 1], F32)
            nc.scalar.activation(actw[:], zc[:], AF.Identity, bias=zc[:])
            nc.scalar.activation(actw[:], zc[:], AF.Relu, bias=zc[:])
            nc.scalar.activation(actw[:], zc[:], AF.Exp, bias=zc[:])
            nc.scalar.copy(actw[:], zc[:])

            # ---- loads: transfers on one queue run sequentially but stripe
            # across the 16 HW DMA engines, so per-queue order = priority and
            # two queues share the ~350 GB/s ring ----
            # conv bias FIRST (tiny): every conv eviction needs it — loading
            # it late blocks Act -> psA fills -> PE stalls
            bct = const.tile([128, 8], F32)
            nc.scalar.dma_start(bct[:], bconv[:, :])
            b1s = const.tile([128, 2], F32)
            nc.scalar.dma_start(b1s[:], b1c[:, :])
            # single gpsimd queue: transfers run strictly in order at the
            # full ring rate, so the order IS the priority — conv-critical
            # pieces first, x-slices for pooling interleaved with the later
            # conv weights (whose deadlines are paced by the conv stream)
            xloc = const.tile([128, HW2], BF16)
            wt = []
            for t in range(8):
                w = const.tile([128, 9 * 128], BF16, tag=f"wt{t}")
                wt.append(w)
            xa = const.tile([128, B * HW2], BF16)

            def load_wt(t):
                nc.gpsimd.dma_start(
                    wt[t][:], wconv[:, t * 9 * 128 : (t + 1) * 9 * 128]
                )

            def load_slice(s):
                nc.gpsimd.dma_start(
                    xa[:, s * HW2 : (s + 1) * HW2], xall[:, s * HW2 : (s + 1) * HW2]
                )

            load_wt(0)
            nc.gpsimd.dma_start(xloc[:, 0 : HW2 // 2], xi[:, 0 : HW2 // 2])
            nc.gpsimd.dma_start(xloc[:, HW2 // 2 :], xi[:, HW2 // 2 :])
            load_wt(1)
            load_wt(2)
            load_slice(0)
            load_slice(1)
            load_wt(3)
            load_slice(2)
            load_slice(3)
            load_wt(4)
            load_slice(4)
            load_slice(5)
            load_wt(5)
            load_slice(6)
            load_slice(7)
            load_wt(6)
            load_wt(7)
            # MLP consts on the scalar queue (concurrent, tiny)
            w1s = const.tile([128, HID], BF16)
            nc.scalar.dma_start(w1s[:], w1t[:, :])
            w2s = const.tile([128, 2 * K], BF16)
            nc.scalar.dma_start(w2s[:], w2t[:, :])
            b2s = const.tile([1, K], BF16)
            nc.scalar.dma_start(b2s[:], b2r[:, :])
            id8 = const.tile([B, B], BF16)
            nc.scalar.dma_start(id8[:], ident8[:, :])
            ones = const.tile([1, B], BF16)
            nc.scalar.dma_start(ones[:], one18[:, :])

            # padded image built on-chip (a strided DMA here would shatter
            # into tiny descriptors and swamp the queues)
            xp = const.tile([128, WP * WP], BF16)
            xp3 = xp[:].rearrange("p (h w) -> p h w", w=WP)
            nc.vector.memset(xp3[:, 0, 0:WP], 0.0)
            nc.vector.memset(xp3[:, WP - 1, 0:WP], 0.0)
            nc.vector.memset(xp3[:, 1 : 1 + HW, 0], 0.0)
            nc.vector.memset(xp3[:, 1 : 1 + HW, WP - 1], 0.0)
            xl3 = xloc[:].rearrange("p (h w) -> p h w", w=HW)
            nc.vector.tensor_copy(
                xp3[:, 1 : 1 + HW // 2, 1 : 1 + HW], xl3[:, 0 : HW // 2, :]
            )
            nc.vector.tensor_copy(
                xp3[:, 1 + HW // 2 : 1 + HW, 1 : 1 + HW], xl3[:, HW // 2 :, :]
            )

            # blend-weight scaffold zeroed early (gpsimd is free after its
            # DMA descriptor burst).  Rows 64-127 replicate rows 0-63 so the
            # lhsT half always shares its base partition with the
            # partition-sliced rhs cs[64u : 64u+64] (PE requirement).
            BD2 = const.tile([128, 128], BF16)
            nc.gpsimd.memset(BD2[:], 0.0)

            # ---- mean-pool all 8 samples (arrival-pipelined on DVE; the
            # tile scheduler slides conv work around the latency) ----
            poolf = const.tile([128, B], F32)
            for s in range(B):
                nc.vector.tensor_reduce(
                    poolf[:, s : s + 1],
                    xa[:, s * HW2 : (s + 1) * HW2],
                    axis=AX.X,
                    op=ALU.add,
                )
            pooled8 = const.tile([128, B], BF16)
            nc.vector.tensor_copy(pooled8[:], poolf[:])

            cs_tiles = [None] * 8

            def emit_conv(t):
                cs = csb_pool.tile([128, HW2], BF16, tag="csb")
                cs_tiles[t] = cs
                for (r0, R) in ROW_GROUPS:
                    pt = psA.tile([128, R * HW], F32, tag="cps")
                    for tap in range(9):
                        dh, dw = divmod(tap, 3)
                        rhs = xp3[:, r0 + dh : r0 + dh + R, dw : dw + HW]
                        nc.tensor.matmul(
                            pt[:],
                            lhsT=wt[t][:, tap * 128 : (tap + 1) * 128],
                            rhs=rhs,
                            start=(tap == 0),
                            stop=(tap == 8),
                        )
                    # PSUM -> SBUF eviction, fused with the conv bias add
                    nc.scalar.activation(
                        cs[:, r0 * HW : (r0 + R) * HW],
                        pt[:],
                        AF.Identity,
                        bias=bct[:, t : t + 1],
                    )

            def emit_blend(t):
                cs = cs_tiles[t]
                for u in range(2):
                    g = 2 * t + u
                    ob = osb_pool.tile([128, HW2], BF16, tag="osb")
                    for ci_, (c0, C) in enumerate(CHUNKS):
                        bp = psB.tile([128, C], F32, tag="bps")
                        nc.tensor.matmul(
                            bp[:],
                            lhsT=BD2[64 * u : 64 * u + 64, :],
                            rhs=cs[64 * u : 64 * u + 64, c0 : c0 + C],
                            start=True,
                            stop=True,
                        )
                        # PSUM drain balanced across DVE and ACT so psB bank
                        # recycling (not one engine) sets the blend rate
                        if ci_ in (1, 4):
                            nc.scalar.copy(ob[:, c0 : c0 + C], bp[:])
                        else:
                            nc.vector.tensor_copy(ob[:, c0 : c0 + C], bp[:])
                    eng = [nc.gpsimd, nc.sync, nc.scalar][g % 3]
                    eng.dma_start(out[:, 16 * g : 16 * g + 16, :], ob[:])

            def emit_mlp():
                # attention MLP + double softmax for all 8 samples at once
                hd = []
                for h in range(2):
                    hps = psM.tile([128, B], F32, tag="mlp")
                    nc.tensor.matmul(
                        hps[:],
                        lhsT=w1s[:, h * 128 : (h + 1) * 128],
                        rhs=pooled8[:],
                        start=True,
                        stop=True,
                    )
                    hsb = const.tile([128, B], BF16, tag=f"hd{h}")
                    nc.scalar.activation(
                        hsb[:], hps[:], AF.Relu, bias=b1s[:, h : h + 1]
                    )
                    hd.append(hsb)

                lps = psM.tile([B, K], F32, tag="mlp")
                nc.tensor.matmul(
                    lps[:], lhsT=hd[0][:], rhs=w2s[:, 0:K], start=True, stop=False
                )
                nc.tensor.matmul(
                    lps[:], lhsT=hd[1][:], rhs=w2s[:, K : 2 * K],
                    start=False, stop=False,
                )
                nc.tensor.matmul(
                    lps[:], lhsT=ones[:], rhs=b2s[:], start=False, stop=True
                )

                # double softmax over k (shift-invariant: max-sub dropped)
                e1 = const.tile([B, K], F32)
                nc.scalar.activation(e1[:], lps[:], AF.Exp, bias=0.0, scale=1.0)
                s1 = const.tile([B, 1], F32)
                nc.vector.tensor_reduce(s1[:], e1[:], axis=AX.X, op=ALU.add)
                r1 = const.tile([B, 1], F32)
                nc.vector.reciprocal(r1[:], s1[:])
                a1 = const.tile([B, K], F32)
                nc.vector.tensor_scalar_mul(a1[:], e1[:], r1[:, 0:1])

                e2 = const.tile([B, K], F32)
                nc.scalar.activation(
                    e2[:], a1[:], AF.Exp, bias=0.0, scale=1.0 / TAU
                )
                s2 = const.tile([B, 1], F32)
                nc.vector.tensor_reduce(s2[:], e2[:], axis=AX.X, op=ALU.add)
                r2 = const.tile([B, 1], F32)
                nc.vector.reciprocal(r2[:], s2[:])
                attn = const.tile([B, K], BF16)
                nc.vector.tensor_scalar_mul(attn[:], e2[:], r2[:, 0:1])

                # attn [j, k] -> attn_T [k, j] via PE transpose
                tps = psM.tile([K, B], BF16, tag="mlp")
                nc.tensor.transpose(tps[:], attn[:], id8[:])
                atT = const.tile([K, B], BF16, tag="atT")
                nc.scalar.copy(atT[:], tps[:])

                # blend weights: BD2[c*4 + k, j*16 + c] = attn[j, k], zero
                # elsewhere (zeros harmless in the 64-contraction).  Compute
                # engines can't start at partition 4c; DMA can — 16 tiny
                # scatters split over two issue queues.
                BDv = BD2[:].rearrange("p (j c) -> p j c", c=16)
                engs = [nc.sync, nc.gpsimd]
                for c in range(16):
                    p0 = c * 4
                    engs[c % 2].dma_start(BDv[p0 : p0 + 4, :, c], atT[:])
                # replicate into the upper partition half for the u=1 lhsT
                nc.sync.dma_start(BD2[64:128, :], BD2[0:64, :])

            # PE stream: blends trail their conv by two tiles, then bunch up
            # before conv7 so only blend7 drains after the last conv — the
            # tile scheduler slides blend matmuls later if BD isn't ready
            emit_conv(0)
            emit_conv(1)
            emit_mlp()
            emit_conv(2)
            for t in range(3, 7):
                emit_blend(t - 3)
                emit_conv(t)
            emit_blend(4)
            emit_blend(5)
            emit_blend(6)
            emit_conv(7)
            emit_blend(7)

    nc.compile()
    return nc


def pack_inputs(x, conv_w, conv_b, w1, b1, w2, b2):
    """Host-side layout packing + bf16 casts (the mean-pool 1/HW^2 scale is
    folded into w1)."""
    bf = ml_dtypes.bfloat16
    x = np.ascontiguousarray(x, dtype=np.float32)
    x_all = x.reshape(B, CIN, HW2).astype(bf)
    xall_T = np.ascontiguousarray(
        x_all.transpose(1, 0, 2).reshape(CIN, B * HW2)
    )

    # conv_w [K, COUT, CIN, 3, 3] -> [ci, t, tap, p] with p = c*4 + k,
    # co = 32 t + c
    w = np.asarray(conv_w, dtype=np.float32).transpose(2, 3, 4, 0, 1)  # ci kh kw k co
    w = w.reshape(CIN, KS, KS, K, 8, 32)  # ci kh kw k t c
    w = w.transpose(0, 4, 1, 2, 5, 3)  # ci t kh kw c k
    wconv = np.ascontiguousarray(w.reshape(CIN, 8 * 9 * 128)).astype(bf)

    bc = np.asarray(conv_b, dtype=np.float32).reshape(K, 8, 32)  # k t c
    bconv = np.ascontiguousarray(bc.transpose(1, 2, 0).reshape(8, 128).T)  # [p, t]

    w1t = (np.ascontiguousarray(np.asarray(w1, dtype=np.float32).T) / float(HW2)).astype(bf)
    b1c = np.ascontiguousarray(np.asarray(b1, dtype=np.float32).reshape(2, 128).T)
    w2T = np.asarray(w2, dtype=np.float32).T  # [256, 4]
    w2t = np.ascontiguousarray(
        np.concatenate([w2T[:128], w2T[128:]], axis=1)
    ).astype(bf)
    b2r = np.asarray(b2, dtype=np.float32).reshape(1, K).astype(bf)
    ident8 = np.eye(B, dtype=np.float32).astype(bf)

    common = dict(
        xall=xall_T, wconv=wconv, bconv=bconv, w1t=w1t, b1c=b1c,
        w2t=w2t, b2r=b2r, ident8=ident8,
        one18=np.ones((1, B), dtype=np.float32).astype(bf),
    )
    in_maps = [
        dict(common, xi=np.ascontiguousarray(x_all[i])) for i in range(NCORES)
    ]
    return in_maps


def run(inputs, trace=False):
    from concourse.bass_utils import run_bass_kernel_spmd

    nc = build_nc()
    in_maps = pack_inputs(**inputs)
    res = run_bass_kernel_spmd(
        nc, in_maps, core_ids=list(range(NCORES)), trace=trace
    )
    slabs = [
        np.asarray(res.results[i]["out"]).astype(np.float32)
        for i in range(NCORES)
    ]
    out = np.stack(slabs, axis=0).reshape(B, B, COUT, HW, HW)
    return out, res


def kernel(**inputs) -> np.ndarray:
    out, _ = run(inputs, trace=False)
    return out
